# revision 6
# baseline (speedup 1.0000x reference)
"""BigBird regressor on 8 trn2 NeuronCores.

Tokens sharded 8 ways for embeddings/LN/projections/FFN (core c owns
blocks [8c,8c+8) of BOTH sequences = 1024 tokens, feature-major layout
[feature, token]). Attention head-sharded via AllToAll: core c handles
seq b=c//4, head group g=c%4 (heads 3g..3g+2) with dense masked
attention — a host-built 0/1 block mask encodes window+global+random
sparsity and the dense global rows, so no runtime gather is needed.
"""
import numpy as np
import ml_dtypes
import concourse.bass as bass
import concourse.mybir as mybir
import concourse.tile as tile
from concourse import bacc
from concourse.bass_utils import run_bass_kernel_spmd
from concourse.masks import make_identity

P = 128
NC = 8
D, H, DH, BLK, L, R = 768, 12, 64, 64, 2, 3
S, B = 4096, 2
NB = S // BLK                     # 64 blocks per seq
TPC = 1024                        # tokens per core (512 per seq)
DF = D // P                       # 6 feature chunks
FF = 4 * D
DFF = FF // P                     # 24
NK = S // P                       # 32 key chunks
QT = S // 512                     # 8 query tiles
FP32 = mybir.dt.float32
BF16 = mybir.dt.bfloat16
AF = mybir.ActivationFunctionType
OP = mybir.AluOpType
SCALE = 1.0 / 8.0


def bcast_free(ap, inner):
    return bass.AP(ap.tensor, ap.offset, list(ap.ap) + [[0, inner]])


def build():
    nc = bacc.Bacc("TRN2", target_bir_lowering=False, debug=False, num_devices=NC)

    emb_in = nc.declare_dram_parameter("embT", [DF, P, TPC], FP32, isOutput=False)
    pos_in = nc.declare_dram_parameter("posT", [DF, P, TPC], FP32, isOutput=False)
    mask_in = nc.declare_dram_parameter("m01", [3, P, NK, NB], BF16, isOutput=False)
    wq_in = nc.declare_dram_parameter("Wq", [L, DF, P, D], BF16, isOutput=False)
    wk_in = nc.declare_dram_parameter("Wk", [L, DF, P, D], BF16, isOutput=False)
    wv_in = nc.declare_dram_parameter("Wv", [L, DF, P, D], BF16, isOutput=False)
    wo_in = nc.declare_dram_parameter("Wo", [L, DF, P, D], BF16, isOutput=False)
    w1_in = nc.declare_dram_parameter("W1", [L, DF, P, FF], BF16, isOutput=False)
    w2_in = nc.declare_dram_parameter("W2", [L, DFF, P, D], BF16, isOutput=False)
    bq_in = nc.declare_dram_parameter("bq", [L, P, DF], FP32, isOutput=False)
    bk_in = nc.declare_dram_parameter("bk", [L, P, DF], FP32, isOutput=False)
    bv_in = nc.declare_dram_parameter("bv", [L, P, DF], FP32, isOutput=False)
    bo_in = nc.declare_dram_parameter("bo", [L, P, DF], FP32, isOutput=False)
    b1_in = nc.declare_dram_parameter("b1", [L, P, DFF], FP32, isOutput=False)
    b2_in = nc.declare_dram_parameter("b2", [L, P, DF], FP32, isOutput=False)
    lng_in = nc.declare_dram_parameter("lng", [5, P, DF], FP32, isOutput=False)
    lnb_in = nc.declare_dram_parameter("lnb", [5, P, DF], FP32, isOutput=False)
    fcw_in = nc.declare_dram_parameter("fcw", [P, DF], BF16, isOutput=False)
    out_par = nc.declare_dram_parameter("partial", [1, B], FP32, isOutput=True)

    with tile.TileContext(nc, num_cores=NC) as tc:
        from contextlib import ExitStack
        es = ExitStack()
        cst = es.enter_context(tc.tile_pool(name="cst", bufs=1))
        big = es.enter_context(tc.tile_pool(name="big", bufs=1))
        buf = es.enter_context(tc.tile_pool(name="buf", bufs=1))   # 32KB-class
        qkvp = es.enter_context(tc.tile_pool(name="qkvp", bufs=1))
        wpool = es.enter_context(tc.tile_pool(name="wp", bufs=2))
        sb = es.enter_context(tc.tile_pool(name="sb", bufs=2))     # small tiles
        psacc = es.enter_context(tc.tile_pool(name="psacc", bufs=2, space="PSUM"))
        psqk = es.enter_context(tc.tile_pool(name="psqk", bufs=2, space="PSUM"))
        psctx = es.enter_context(tc.tile_pool(name="psctx", bufs=1, space="PSUM"))
        psmisc = es.enter_context(tc.tile_pool(name="psmisc", bufs=1, space="PSUM"))
        dram = es.enter_context(tc.tile_pool(name="dram", bufs=1, space="DRAM"))

        ones_c = cst.tile([P, 1], BF16)
        nc.any.memset(ones_c[:], 1.0)
        ones_r = cst.tile([1, P], BF16)
        nc.any.memset(ones_r[:], 1.0)
        ident = cst.tile([64, 64], BF16)
        make_identity(nc, ident)

        lng = cst.tile([P, 5, DF], FP32)
        nc.sync.dma_start(lng[:], lng_in[:].rearrange("a p c -> p a c"))
        lnb = cst.tile([P, 5, DF], FP32)
        nc.sync.dma_start(lnb[:], lnb_in[:].rearrange("a p c -> p a c"))
        m01 = cst.tile([P, 3, NK, NB], BF16)
        nc.sync.dma_start(m01[:], mask_in[:].rearrange("h p k b -> p h k b"))

        x = big.tile([P, DF, TPC], FP32)     # residual stream fp32
        xb = big.tile([P, DF, TPC], BF16)    # ln output bf16

        # x = embT + posT
        emb = buf.tile([P, DF, TPC], FP32, tag="buf1")
        nc.sync.dma_start(emb[:], emb_in[:].rearrange("a p c -> p a c"))
        for half in range(2):
            hs = slice(512 * half, 512 * (half + 1))
            pos = buf.tile([P, DF, 512], FP32, tag="buf2")
            nc.sync.dma_start(pos[:], pos_in[:, :, hs].rearrange("a p c -> p a c"))
            for f in range(DF):
                nc.vector.tensor_tensor(out=x[:, f, hs], in0=emb[:, f, hs],
                                        in1=pos[:, f], op=OP.add)

        def layernorm(ln_idx):
            """x (fp32) -> xb (bf16, normalized*g+b)."""
            for f in range(DF):
                nc.scalar.copy(xb[:, f], x[:, f])
            x2 = buf.tile([P, DF, TPC], BF16, tag="buf2")
            for f in range(DF):
                nc.vector.tensor_tensor(out=x2[:, f], in0=x[:, f], in1=x[:, f],
                                        op=OP.mult)
            for tt in range(2):
                sl = slice(512 * tt, 512 * (tt + 1))
                pm = psmisc.tile([1, 512], FP32, tag="pm")
                for f in range(DF):
                    nc.tensor.matmul(pm[:], ones_c[:], xb[:, f, sl],
                                     start=(f == 0), stop=(f == DF - 1))
                pv = psmisc.tile([1, 512], FP32, tag="pv")
                for f in range(DF):
                    nc.tensor.matmul(pv[:], ones_c[:], x2[:, f, sl],
                                     start=(f == 0), stop=(f == DF - 1))
                mr = sb.tile([1, 512], FP32, tag="mr")
                nc.scalar.mul(mr[:], pm[:], 1.0 / D)
                vr = sb.tile([1, 512], FP32, tag="vr")
                nc.scalar.mul(vr[:], pv[:], 1.0 / D)
                m2 = sb.tile([1, 512], FP32, tag="m2")
                nc.vector.tensor_tensor(out=m2[:], in0=mr[:], in1=mr[:],
                                        op=OP.mult)
                nc.vector.tensor_tensor(out=vr[:], in0=vr[:], in1=m2[:],
                                        op=OP.subtract)
                sd = sb.tile([1, 512], FP32, tag="sd")
                nc.scalar.sqrt(sd[:], vr[:])
                rs = sb.tile([1, 512], FP32, tag="rs")
                nc.vector.reciprocal(rs[:], sd[:])
                mrb = sb.tile([1, 512], BF16, tag="mrb")
                nc.scalar.copy(mrb[:], mr[:])
                rsb = sb.tile([1, 512], BF16, tag="rsb")
                nc.scalar.copy(rsb[:], rs[:])
                mB = psmisc.tile([P, 512], FP32, tag="mB")
                nc.tensor.matmul(mB[:], ones_r[:], mrb[:], start=True, stop=True)
                for f in range(DF):
                    nc.vector.tensor_tensor(out=x[:, f, sl], in0=x[:, f, sl],
                                            in1=mB[:], op=OP.subtract)
                rB = psmisc.tile([P, 512], FP32, tag="mB")
                nc.tensor.matmul(rB[:], ones_r[:], rsb[:], start=True, stop=True)
                for f in range(DF):
                    nc.vector.tensor_tensor(out=x[:, f, sl], in0=x[:, f, sl],
                                            in1=rB[:], op=OP.mult)
                    # post-LN: residual stream becomes the LN output
                    nc.scalar.activation(x[:, f, sl], x[:, f, sl], AF.Identity,
                                         bias=lnb[:, ln_idx, f:f + 1],
                                         scale=lng[:, ln_idx, f:f + 1])
                    nc.scalar.copy(xb[:, f, sl], x[:, f, sl])

        def matmul_block(w_dram, nout_chunks, src, evict):
            nin = src.shape[1]
            for fc in range(nout_chunks):
                w = wpool.tile([P, nin, P], BF16, tag="w")
                nc.sync.dma_start(
                    w[:], w_dram[:, :, 128 * fc:128 * (fc + 1)]
                    .rearrange("a p c -> p a c"))
                for tt in range(2):
                    pt = psacc.tile([P, 512], FP32, tag="pt")
                    for kc in range(nin):
                        nc.tensor.matmul(pt[:], w[:, kc],
                                         src[:, kc, 512 * tt:512 * (tt + 1)],
                                         start=(kc == 0), stop=(kc == nin - 1))
                    evict(fc, tt, pt)

        layernorm(0)

        for l in range(L):
            # ---------- QKV projection -> A2A shards ----------
            qkv_in = dram.tile([NC, 576, 512], BF16, tag="qkv_in")
            qkv_out = dram.tile([NC, 576, 512], BF16, tag="qkv_out")
            bqkv = sb.tile([P, 3, DF], FP32, tag="bqkv")
            nc.sync.dma_start(bqkv[:, 0], bq_in[l])
            nc.sync.dma_start(bqkv[:, 1], bk_in[l])
            nc.sync.dma_start(bqkv[:, 2], bv_in[l])

            for p_i, w_dram in enumerate((wq_in, wk_in, wv_in)):
                def ev_qkv(fc, tt, pt, p_i=p_i):
                    st = sb.tile([P, 512], BF16, tag="stq")
                    nc.scalar.activation(st[:], pt[:], AF.Identity,
                                         bias=bqkv[:, p_i, fc:fc + 1])
                    for u in range(2):
                        h = 2 * fc + u
                        j = 4 * tt + h // 3
                        row = 192 * p_i + 64 * (h % 3)
                        nc.scalar.dma_start(
                            qkv_in[j, row:row + 64, :],
                            st[64 * u:64 * u + 64, :])
                matmul_block(w_dram[l], DF, xb, ev_qkv)

            nc.gpsimd.collective_compute(
                "AllToAll", OP.bypass, replica_groups=[list(range(NC))],
                ins=[qkv_in[:]], outs=[qkv_out[:]])

            # ---------- dense masked attention ----------
            ctx_in = dram.tile([NC, 192, 512], BF16, tag="ctx_in")
            ctx_out = dram.tile([NC, 192, 512], BF16, tag="ctx_out")
            for hl in range(3):
                qT = qkvp.tile([64, NK, P], BF16, tag="qT")
                kT = qkvp.tile([64, NK, P], BF16, tag="kT")
                vT = qkvp.tile([64, NK, P], BF16, tag="vT")
                for t, sec in ((qT, 0), (kT, 192), (vT, 384)):
                    for s in range(NC):
                        nc.sync.dma_start(
                            t[:, 4 * s:4 * s + 4, :],
                            qkv_out[s, sec + 64 * hl: sec + 64 * (hl + 1), :]
                            .rearrange("p (c q) -> p c q", q=128))
                vaug = qkvp.tile([P, NK, 65], BF16, tag="vaug")
                nc.any.memset(vaug[:, :, 64:65], 1.0)
                for ck in range(NK):
                    vps = psqk.tile([P, 64], BF16, tag="sps")
                    nc.tensor.transpose(vps[:], vT[:, ck, :], ident[:])
                    nc.scalar.copy(vaug[:, ck, 0:64], vps[:])
                for qt in range(QT):
                    a_sb = buf.tile([P, NK, 512], BF16, tag="buf1")
                    for ck in range(NK):
                        sps = psqk.tile([P, 512], FP32, tag="sps")
                        nc.tensor.matmul(sps[:], kT[:, ck, :],
                                         qT[:, 4 * qt:4 * qt + 4, :],
                                         start=True, stop=True)
                        nc.scalar.activation(a_sb[:, ck], sps[:], AF.Exp,
                                             scale=SCALE)
                        mk = m01[:, hl, ck, 8 * qt:8 * qt + 8]
                        av = a_sb[:, ck].rearrange("p (b t) -> p b t", t=64)
                        nc.vector.tensor_tensor(out=av, in0=av,
                                                in1=bcast_free(mk, 64),
                                                op=OP.mult)
                    cps = psctx.tile([65, 512], FP32, tag="cps")
                    for ck in range(NK):
                        nc.tensor.matmul(cps[:], vaug[:, ck, :], a_sb[:, ck],
                                         start=(ck == 0), stop=(ck == NK - 1))
                    rc = sb.tile([1, 512], FP32, tag="rc")
                    nc.vector.reciprocal(rc[:], cps[64:65, :])
                    rcb = sb.tile([1, 512], BF16, tag="rcb")
                    nc.scalar.copy(rcb[:], rc[:])
                    rB2 = psmisc.tile([64, 512], FP32, tag="mB")
                    nc.tensor.matmul(rB2[:], ones_r[:, 0:64], rcb[:],
                                     start=True, stop=True)
                    cu = sb.tile([64, 512], FP32, tag="cu")
                    nc.scalar.copy(cu[:], cps[0:64, :])
                    cn = sb.tile([64, 512], BF16, tag="cn")
                    nc.vector.tensor_tensor(out=cn[:], in0=cu[:],
                                            in1=rB2[:], op=OP.mult)
                    nc.scalar.dma_start(
                        ctx_in[qt, 64 * hl:64 * (hl + 1), :], cn[:])

            nc.gpsimd.collective_compute(
                "AllToAll", OP.bypass, replica_groups=[list(range(NC))],
                ins=[ctx_in[:]], outs=[ctx_out[:]])

            ctxf = buf.tile([P, DF, TPC], BF16, tag="buf2")
            for j in range(NC):
                r0, c0 = 192 * (j % 4), 512 * (j // 4)
                r = r0
                while r < r0 + 192:
                    f, off = divmod(r, P)
                    take = min(P - off, r0 + 192 - r)
                    nc.sync.dma_start(
                        ctxf[off:off + take, f, c0:c0 + 512],
                        ctx_out[j, r - r0:r - r0 + take, :])
                    r += take

            # ---------- O proj + residual + ln1 ----------
            bot = sb.tile([P, DF, 2], FP32, tag="bot")
            nc.sync.dma_start(bot[:, :, 0], bo_in[l])
            nc.sync.dma_start(bot[:, :, 1], b2_in[l])

            def ev_o(fc, tt, pt):
                st = sb.tile([P, 512], FP32, tag="sto")
                nc.scalar.activation(st[:], pt[:], AF.Identity,
                                     bias=bot[:, fc, 0:1])
                sl = slice(512 * tt, 512 * (tt + 1))
                nc.vector.tensor_tensor(out=x[:, fc, sl], in0=x[:, fc, sl],
                                        in1=st[:], op=OP.add)
            matmul_block(wo_in[l], DF, ctxf, ev_o)
            layernorm(1 + 2 * l)

            # ---------- FFN (token-half at a time) ----------
            b1t = sb.tile([P, DFF], FP32, tag="b1t")
            nc.sync.dma_start(b1t[:], b1_in[l])

            for tt in range(2):
                sl = slice(512 * tt, 512 * (tt + 1))
                h1 = big.tile([P, DFF, 512], BF16, tag="h1")
                for fc in range(DFF):
                    w = wpool.tile([P, DF, P], BF16, tag="w")
                    nc.sync.dma_start(
                        w[:], w1_in[l, :, :, 128 * fc:128 * (fc + 1)]
                        .rearrange("a p c -> p a c"))
                    pt = psacc.tile([P, 512], FP32, tag="pt")
                    for kc in range(DF):
                        nc.tensor.matmul(pt[:], w[:, kc], xb[:, kc, sl],
                                         start=(kc == 0), stop=(kc == DF - 1))
                    nc.scalar.activation(h1[:, fc], pt[:], AF.Gelu,
                                         bias=b1t[:, fc:fc + 1])
                for fc in range(DF):
                    w = wpool.tile([P, DFF, P], BF16, tag="w")
                    nc.sync.dma_start(
                        w[:], w2_in[l, :, :, 128 * fc:128 * (fc + 1)]
                        .rearrange("a p c -> p a c"))
                    pt = psacc.tile([P, 512], FP32, tag="pt")
                    for kc in range(DFF):
                        nc.tensor.matmul(pt[:], w[:, kc], h1[:, kc],
                                         start=(kc == 0), stop=(kc == DFF - 1))
                    st = sb.tile([P, 512], FP32, tag="sto")
                    nc.scalar.activation(st[:], pt[:], AF.Identity,
                                         bias=bot[:, fc, 1:2])
                    nc.vector.tensor_tensor(out=x[:, fc, sl], in0=x[:, fc, sl],
                                            in1=st[:], op=OP.add)
            layernorm(2 + 2 * l)

        # ---------- pool + fc ----------
        fcw = cst.tile([P, DF], BF16)
        nc.sync.dma_start(fcw[:], fcw_in[:])
        pooled = sb.tile([P, DF, B], FP32, tag="pooled")
        for f in range(DF):
            for b in range(B):
                scr = sb.tile([P, 512], FP32, tag="scr")
                nc.scalar.activation(scr[:], x[:, f, 512 * b:512 * (b + 1)],
                                     AF.Copy, accum_out=pooled[:, f, b:b + 1])
        poolb = sb.tile([P, DF, B], BF16, tag="poolb")
        nc.scalar.copy(poolb[:], pooled[:])
        pp = psmisc.tile([1, B], FP32, tag="pm")
        for f in range(DF):
            nc.tensor.matmul(pp[:], fcw[:, f:f + 1], poolb[:, f],
                             start=(f == 0), stop=(f == DF - 1))
        outt = sb.tile([1, B], FP32, tag="outt")
        nc.scalar.copy(outt[:], pp[:])
        nc.scalar.dma_start(out_par[:], outt[:])
        es.close()
    nc.finalize()
    return nc


_CACHED = {}


def kernel(**inputs):
    inputs = {k: np.asarray(v) for k, v in inputs.items()}
    ids = np.asarray(inputs["input_ids"])
    rand_blocks = np.asarray(inputs["rand_blocks"])
    bf = lambda a: np.ascontiguousarray(a).astype(ml_dtypes.bfloat16)
    f32 = lambda a: np.ascontiguousarray(a, dtype=np.float32)

    n = NB
    rows = np.arange(n)
    allowed = np.zeros((H, n, n), bool)
    for h in range(H):
        allowed[h, rows, 0] = True
        allowed[h, rows, n - 1] = True
        allowed[h, rows, np.clip(rows - 1, 0, n - 1)] = True
        allowed[h, rows, rows] = True
        allowed[h, rows, np.clip(rows + 1, 0, n - 1)] = True
        for r in range(R):
            allowed[h, rows, rand_blocks[h, :, r]] = True
    allowed[:, 0, :] = True
    allowed[:, n - 1, :] = True

    emb = np.asarray(inputs["emb_tok"])[ids]      # [2, 4096, 768]
    pos = np.asarray(inputs["emb_pos"])
    kblk = np.arange(S) // BLK

    shared = {"fcw": bf(np.asarray(inputs["fc_w"])[:, 0].reshape(DF, P).T)}
    for nm in ("Wq", "Wk", "Wv", "Wo"):
        shared[nm] = bf(inputs[nm].reshape(L, DF, P, D))
    shared["W1"] = bf(inputs["W1"].reshape(L, DF, P, FF))
    shared["W2"] = bf(inputs["W2"].reshape(L, DFF, P, D))
    for nm in ("bq", "bk", "bv", "bo", "b2"):
        shared[nm] = f32(inputs[nm].reshape(L, DF, P).transpose(0, 2, 1))
    shared["b1"] = f32(inputs["b1"].reshape(L, DFF, P).transpose(0, 2, 1))
    lng = np.stack([inputs["ln_emb_g"], inputs["ln1_g"][0], inputs["ln2_g"][0],
                    inputs["ln1_g"][1], inputs["ln2_g"][1]])
    lnb = np.stack([inputs["ln_emb_b"], inputs["ln1_b"][0], inputs["ln2_b"][0],
                    inputs["ln1_b"][1], inputs["ln2_b"][1]])
    shared["lng"] = f32(lng.reshape(5, DF, P).transpose(0, 2, 1))
    shared["lnb"] = f32(lnb.reshape(5, DF, P).transpose(0, 2, 1))

    in_maps = []
    for c in range(NC):
        g = c % 4
        sl = slice(512 * c, 512 * (c + 1))
        embT = f32(np.concatenate([emb[0, sl], emb[1, sl]], 0).T
                   .reshape(DF, P, TPC))
        posT = f32(np.concatenate([pos[sl], pos[sl]], 0).T.reshape(DF, P, TPC))
        m = allowed[3 * g:3 * g + 3][:, :, kblk]          # [3, ib, k]
        m01 = bf(m.transpose(0, 2, 1).reshape(3, NK, P, NB)
                 .transpose(0, 2, 1, 3))
        in_maps.append({"embT": embT, "posT": posT, "m01": m01, **shared})

    if "nc" not in _CACHED:
        _CACHED["nc"] = build()
    res = run_bass_kernel_spmd(_CACHED["nc"], in_maps, list(range(NC))).results
    partial = np.stack([r["partial"][0] for r in res])
    out = partial.sum(0) / S + np.asarray(inputs["fc_b"])[0]
    return out.astype(np.float32)


# revision 8
# speedup vs baseline: 2698.7669x; 2698.7669x over previous
"""BigBird regressor on 8 trn2 NeuronCores.

Tokens sharded 8 ways for embeddings/LN/projections/FFN (core c owns
blocks [8c,8c+8) of BOTH sequences = 1024 tokens, feature-major layout
[feature, token]). Attention head-sharded via AllToAll: core c handles
seq b=c//4, head group g=c%4 (heads 3g..3g+2) with dense masked
attention — a host-built 0/1 block mask encodes window+global+random
sparsity and the dense global rows, so no runtime gather is needed.
"""
import numpy as np
import ml_dtypes
import concourse.bass as bass
import concourse.mybir as mybir
import concourse.tile as tile
from concourse import bacc
from concourse.bass_utils import run_bass_kernel_spmd
from concourse.masks import make_identity

P = 128
NC = 8
D, H, DH, BLK, L, R = 768, 12, 64, 64, 2, 3
S, B = 4096, 2
NB = S // BLK                     # 64 blocks per seq
TPC = 1024                        # tokens per core (512 per seq)
DF = D // P                       # 6 feature chunks
FF = 4 * D
DFF = FF // P                     # 24
NK = S // P                       # 32 key chunks
QT = S // 512                     # 8 query tiles
FP32 = mybir.dt.float32
BF16 = mybir.dt.bfloat16
AF = mybir.ActivationFunctionType
OP = mybir.AluOpType
SCALE = 1.0 / 8.0


def bcast_free(ap, inner):
    return bass.AP(ap.tensor, ap.offset, list(ap.ap) + [[0, inner]])


def build():
    nc = bacc.Bacc("TRN2", target_bir_lowering=False, debug=False, num_devices=NC)

    emb_in = nc.declare_dram_parameter("embT", [DF, P, TPC], FP32, isOutput=False)
    pos_in = nc.declare_dram_parameter("posT", [DF, P, TPC], FP32, isOutput=False)
    mask_in = nc.declare_dram_parameter("m01", [3, P, NK, NB], BF16, isOutput=False)
    wq_in = nc.declare_dram_parameter("Wq", [L, DF, P, D], BF16, isOutput=False)
    wk_in = nc.declare_dram_parameter("Wk", [L, DF, P, D], BF16, isOutput=False)
    wv_in = nc.declare_dram_parameter("Wv", [L, DF, P, D], BF16, isOutput=False)
    wo_in = nc.declare_dram_parameter("Wo", [L, DF, P, D], BF16, isOutput=False)
    w1_in = nc.declare_dram_parameter("W1", [L, DF, P, FF], BF16, isOutput=False)
    w2_in = nc.declare_dram_parameter("W2", [L, DFF, P, D], BF16, isOutput=False)
    bq_in = nc.declare_dram_parameter("bq", [L, P, DF], FP32, isOutput=False)
    bk_in = nc.declare_dram_parameter("bk", [L, P, DF], FP32, isOutput=False)
    bv_in = nc.declare_dram_parameter("bv", [L, P, DF], FP32, isOutput=False)
    bo_in = nc.declare_dram_parameter("bo", [L, P, DF], FP32, isOutput=False)
    b1_in = nc.declare_dram_parameter("b1", [L, P, DFF], FP32, isOutput=False)
    b2_in = nc.declare_dram_parameter("b2", [L, P, DF], FP32, isOutput=False)
    lng_in = nc.declare_dram_parameter("lng", [5, P, DF], FP32, isOutput=False)
    lnb_in = nc.declare_dram_parameter("lnb", [5, P, DF], FP32, isOutput=False)
    fcw_in = nc.declare_dram_parameter("fcw", [P, DF], BF16, isOutput=False)
    out_par = nc.declare_dram_parameter("partial", [1, B], FP32, isOutput=True)

    with tile.TileContext(nc, num_cores=NC) as tc:
        from contextlib import ExitStack
        es = ExitStack()
        cst = es.enter_context(tc.tile_pool(name="cst", bufs=1))
        big = es.enter_context(tc.tile_pool(name="big", bufs=1))
        buf = es.enter_context(tc.tile_pool(name="buf", bufs=1))   # 32KB-class
        qkvp = es.enter_context(tc.tile_pool(name="qkvp", bufs=1))
        wpool = es.enter_context(tc.tile_pool(name="wp", bufs=2))
        sb = es.enter_context(tc.tile_pool(name="sb", bufs=2))     # small tiles
        psacc = es.enter_context(tc.tile_pool(name="psacc", bufs=2, space="PSUM"))
        psqk = es.enter_context(tc.tile_pool(name="psqk", bufs=2, space="PSUM"))
        psctx = es.enter_context(tc.tile_pool(name="psctx", bufs=1, space="PSUM"))
        psmisc = es.enter_context(tc.tile_pool(name="psmisc", bufs=1, space="PSUM"))
        dram = es.enter_context(tc.tile_pool(name="dram", bufs=1, space="DRAM"))

        ones_c = cst.tile([P, 1], BF16)
        nc.any.memset(ones_c[:], 1.0)
        ones_r = cst.tile([1, P], BF16)
        nc.any.memset(ones_r[:], 1.0)
        ident = cst.tile([64, 64], BF16)
        make_identity(nc, ident)

        lng = cst.tile([P, 5, DF], FP32)
        nc.sync.dma_start(lng[:], lng_in[:].rearrange("a p c -> p a c"))
        lnb = cst.tile([P, 5, DF], FP32)
        nc.sync.dma_start(lnb[:], lnb_in[:].rearrange("a p c -> p a c"))
        m01 = cst.tile([P, 3, NK, NB], BF16)
        nc.sync.dma_start(m01[:], mask_in[:].rearrange("h p k b -> p h k b"))

        x = big.tile([P, DF, TPC], FP32)     # residual stream fp32
        xb = big.tile([P, DF, TPC], BF16)    # ln output bf16

        # x = embT + posT
        emb = buf.tile([P, DF, TPC], FP32, tag="buf1")
        nc.sync.dma_start(emb[:], emb_in[:].rearrange("a p c -> p a c"))
        for half in range(2):
            hs = slice(512 * half, 512 * (half + 1))
            pos = buf.tile([P, DF, 512], FP32, tag="buf2")
            nc.sync.dma_start(pos[:], pos_in[:, :, hs].rearrange("a p c -> p a c"))
            for f in range(DF):
                nc.vector.tensor_tensor(out=x[:, f, hs], in0=emb[:, f, hs],
                                        in1=pos[:, f], op=OP.add)

        def layernorm(ln_idx):
            """x (fp32) -> xb (bf16, normalized*g+b)."""
            for f in range(DF):
                nc.scalar.copy(xb[:, f], x[:, f])
            x2 = buf.tile([P, DF, TPC], BF16, tag="buf2")
            for f in range(DF):
                nc.vector.tensor_tensor(out=x2[:, f], in0=x[:, f], in1=x[:, f],
                                        op=OP.mult)
            for tt in range(2):
                sl = slice(512 * tt, 512 * (tt + 1))
                pm = psmisc.tile([1, 512], FP32, tag="pm")
                for f in range(DF):
                    nc.tensor.matmul(pm[:], ones_c[:], xb[:, f, sl],
                                     start=(f == 0), stop=(f == DF - 1))
                pv = psmisc.tile([1, 512], FP32, tag="pv")
                for f in range(DF):
                    nc.tensor.matmul(pv[:], ones_c[:], x2[:, f, sl],
                                     start=(f == 0), stop=(f == DF - 1))
                mr = sb.tile([1, 512], FP32, tag="mr")
                nc.scalar.mul(mr[:], pm[:], 1.0 / D)
                vr = sb.tile([1, 512], FP32, tag="vr")
                nc.scalar.mul(vr[:], pv[:], 1.0 / D)
                m2 = sb.tile([1, 512], FP32, tag="m2")
                nc.vector.tensor_tensor(out=m2[:], in0=mr[:], in1=mr[:],
                                        op=OP.mult)
                nc.vector.tensor_tensor(out=vr[:], in0=vr[:], in1=m2[:],
                                        op=OP.subtract)
                sd = sb.tile([1, 512], FP32, tag="sd")
                nc.scalar.sqrt(sd[:], vr[:])
                rs = sb.tile([1, 512], FP32, tag="rs")
                nc.vector.reciprocal(rs[:], sd[:])
                mrb = sb.tile([1, 512], BF16, tag="mrb")
                nc.scalar.copy(mrb[:], mr[:])
                rsb = sb.tile([1, 512], BF16, tag="rsb")
                nc.scalar.copy(rsb[:], rs[:])
                mB = psmisc.tile([P, 512], FP32, tag="mB")
                nc.tensor.matmul(mB[:], ones_r[:], mrb[:], start=True, stop=True)
                for f in range(DF):
                    nc.vector.tensor_tensor(out=x[:, f, sl], in0=x[:, f, sl],
                                            in1=mB[:], op=OP.subtract)
                rB = psmisc.tile([P, 512], FP32, tag="mB")
                nc.tensor.matmul(rB[:], ones_r[:], rsb[:], start=True, stop=True)
                for f in range(DF):
                    nc.vector.tensor_tensor(out=x[:, f, sl], in0=x[:, f, sl],
                                            in1=rB[:], op=OP.mult)
                    # post-LN: residual stream becomes the LN output
                    nc.scalar.activation(x[:, f, sl], x[:, f, sl], AF.Identity,
                                         bias=lnb[:, ln_idx, f:f + 1],
                                         scale=lng[:, ln_idx, f:f + 1])
                    nc.scalar.copy(xb[:, f, sl], x[:, f, sl])

        def matmul_block(w_dram, nout_chunks, src, evict):
            nin = src.shape[1]
            for fc in range(nout_chunks):
                w = wpool.tile([P, nin, P], BF16, tag="w")
                nc.sync.dma_start(
                    w[:], w_dram[:, :, 128 * fc:128 * (fc + 1)]
                    .rearrange("a p c -> p a c"))
                for tt in range(2):
                    pt = psacc.tile([P, 512], FP32, tag="pt")
                    for kc in range(nin):
                        nc.tensor.matmul(pt[:], w[:, kc],
                                         src[:, kc, 512 * tt:512 * (tt + 1)],
                                         start=(kc == 0), stop=(kc == nin - 1))
                    evict(fc, tt, pt)

        layernorm(0)

        for l in range(L):
            # ---------- QKV projection -> A2A shards ----------
            qkv_in = dram.tile([NC, 576, 512], BF16, tag="qkv_in")
            qkv_out = dram.tile([NC, 576, 512], BF16, tag="qkv_out")
            bqkv = sb.tile([P, 3, DF], FP32, tag="bqkv")
            nc.sync.dma_start(bqkv[:, 0], bq_in[l])
            nc.sync.dma_start(bqkv[:, 1], bk_in[l])
            nc.sync.dma_start(bqkv[:, 2], bv_in[l])

            for p_i, w_dram in enumerate((wq_in, wk_in, wv_in)):
                def ev_qkv(fc, tt, pt, p_i=p_i):
                    st = sb.tile([P, 512], BF16, tag="stq")
                    nc.scalar.activation(st[:], pt[:], AF.Identity,
                                         bias=bqkv[:, p_i, fc:fc + 1])
                    for u in range(2):
                        h = 2 * fc + u
                        j = 4 * tt + h // 3
                        row = 192 * p_i + 64 * (h % 3)
                        nc.scalar.dma_start(
                            qkv_in[j, row:row + 64, :],
                            st[64 * u:64 * u + 64, :])
                matmul_block(w_dram[l], DF, xb, ev_qkv)

            nc.gpsimd.collective_compute(
                "AllToAll", OP.bypass, replica_groups=[list(range(NC))],
                ins=[qkv_in[:]], outs=[qkv_out[:]])

            # ---------- dense masked attention ----------
            ctx_in = dram.tile([NC, 192, 512], BF16, tag="ctx_in")
            ctx_out = dram.tile([NC, 192, 512], BF16, tag="ctx_out")
            for hl in range(3):
                qT = qkvp.tile([64, NK, P], BF16, tag="qT")
                kT = qkvp.tile([64, NK, P], BF16, tag="kT")
                vT = qkvp.tile([64, NK, P], BF16, tag="vT")
                for t, sec in ((qT, 0), (kT, 192), (vT, 384)):
                    for s in range(NC):
                        nc.sync.dma_start(
                            t[:, 4 * s:4 * s + 4, :],
                            qkv_out[s, sec + 64 * hl: sec + 64 * (hl + 1), :]
                            .rearrange("p (c q) -> p c q", q=128))
                vaug = qkvp.tile([P, NK, 65], BF16, tag="vaug")
                nc.any.memset(vaug[:, :, 64:65], 1.0)
                for ck in range(NK):
                    vps = psqk.tile([P, 64], BF16, tag="sps")
                    nc.tensor.transpose(vps[:], vT[:, ck, :], ident[:])
                    nc.scalar.copy(vaug[:, ck, 0:64], vps[:])
                for qt in range(QT):
                    a_sb = buf.tile([P, NK, 512], BF16, tag="buf1")
                    for ck in range(NK):
                        sps = psqk.tile([P, 512], FP32, tag="sps")
                        nc.tensor.matmul(sps[:], kT[:, ck, :],
                                         qT[:, 4 * qt:4 * qt + 4, :],
                                         start=True, stop=True)
                        nc.scalar.activation(a_sb[:, ck], sps[:], AF.Exp,
                                             scale=SCALE)
                        mk = m01[:, hl, ck, 8 * qt:8 * qt + 8]
                        av = a_sb[:, ck].rearrange("p (b t) -> p b t", t=64)
                        nc.vector.tensor_tensor(out=av, in0=av,
                                                in1=bcast_free(mk, 64),
                                                op=OP.mult)
                    cps = psctx.tile([65, 512], FP32, tag="cps")
                    for ck in range(NK):
                        nc.tensor.matmul(cps[:], vaug[:, ck, :], a_sb[:, ck],
                                         start=(ck == 0), stop=(ck == NK - 1))
                    rc = sb.tile([1, 512], FP32, tag="rc")
                    nc.vector.reciprocal(rc[:], cps[64:65, :])
                    rcb = sb.tile([1, 512], BF16, tag="rcb")
                    nc.scalar.copy(rcb[:], rc[:])
                    rB2 = psmisc.tile([64, 512], FP32, tag="mB")
                    nc.tensor.matmul(rB2[:], ones_r[:, 0:64], rcb[:],
                                     start=True, stop=True)
                    cu = sb.tile([64, 512], FP32, tag="cu")
                    nc.scalar.copy(cu[:], cps[0:64, :])
                    cn = sb.tile([64, 512], BF16, tag="cn")
                    nc.vector.tensor_tensor(out=cn[:], in0=cu[:],
                                            in1=rB2[:], op=OP.mult)
                    nc.scalar.dma_start(
                        ctx_in[qt, 64 * hl:64 * (hl + 1), :], cn[:])

            nc.gpsimd.collective_compute(
                "AllToAll", OP.bypass, replica_groups=[list(range(NC))],
                ins=[ctx_in[:]], outs=[ctx_out[:]])

            ctxf = buf.tile([P, DF, TPC], BF16, tag="buf2")
            for j in range(NC):
                r0, c0 = 192 * (j % 4), 512 * (j // 4)
                r = r0
                while r < r0 + 192:
                    f, off = divmod(r, P)
                    take = min(P - off, r0 + 192 - r)
                    nc.sync.dma_start(
                        ctxf[off:off + take, f, c0:c0 + 512],
                        ctx_out[j, r - r0:r - r0 + take, :])
                    r += take

            # ---------- O proj + residual + ln1 ----------
            bot = sb.tile([P, DF, 2], FP32, tag="bot")
            nc.sync.dma_start(bot[:, :, 0], bo_in[l])
            nc.sync.dma_start(bot[:, :, 1], b2_in[l])

            def ev_o(fc, tt, pt):
                st = sb.tile([P, 512], FP32, tag="sto")
                nc.scalar.activation(st[:], pt[:], AF.Identity,
                                     bias=bot[:, fc, 0:1])
                sl = slice(512 * tt, 512 * (tt + 1))
                nc.vector.tensor_tensor(out=x[:, fc, sl], in0=x[:, fc, sl],
                                        in1=st[:], op=OP.add)
            matmul_block(wo_in[l], DF, ctxf, ev_o)
            layernorm(1 + 2 * l)

            # ---------- FFN (token-half at a time) ----------
            b1t = sb.tile([P, DFF], FP32, tag="b1t")
            nc.sync.dma_start(b1t[:], b1_in[l])

            for tt in range(2):
                sl = slice(512 * tt, 512 * (tt + 1))
                h1 = big.tile([P, DFF, 512], BF16, tag="h1")
                for fc in range(DFF):
                    w = wpool.tile([P, DF, P], BF16, tag="w")
                    nc.sync.dma_start(
                        w[:], w1_in[l, :, :, 128 * fc:128 * (fc + 1)]
                        .rearrange("a p c -> p a c"))
                    pt = psacc.tile([P, 512], FP32, tag="pt")
                    for kc in range(DF):
                        nc.tensor.matmul(pt[:], w[:, kc], xb[:, kc, sl],
                                         start=(kc == 0), stop=(kc == DF - 1))
                    nc.scalar.activation(h1[:, fc], pt[:], AF.Gelu,
                                         bias=b1t[:, fc:fc + 1])
                for fc in range(DF):
                    w = wpool.tile([P, DFF, P], BF16, tag="w")
                    nc.sync.dma_start(
                        w[:], w2_in[l, :, :, 128 * fc:128 * (fc + 1)]
                        .rearrange("a p c -> p a c"))
                    pt = psacc.tile([P, 512], FP32, tag="pt")
                    for kc in range(DFF):
                        nc.tensor.matmul(pt[:], w[:, kc], h1[:, kc],
                                         start=(kc == 0), stop=(kc == DFF - 1))
                    st = sb.tile([P, 512], FP32, tag="sto")
                    nc.scalar.activation(st[:], pt[:], AF.Identity,
                                         bias=bot[:, fc, 1:2])
                    nc.vector.tensor_tensor(out=x[:, fc, sl], in0=x[:, fc, sl],
                                            in1=st[:], op=OP.add)
            layernorm(2 + 2 * l)

        # ---------- pool + fc ----------
        fcw = cst.tile([P, DF], BF16)
        nc.sync.dma_start(fcw[:], fcw_in[:])
        pooled = sb.tile([P, DF, B], FP32, tag="pooled")
        for f in range(DF):
            for b in range(B):
                scr = sb.tile([P, 512], FP32, tag="scr")
                nc.scalar.activation(scr[:], x[:, f, 512 * b:512 * (b + 1)],
                                     AF.Copy, accum_out=pooled[:, f, b:b + 1])
        poolb = sb.tile([P, DF, B], BF16, tag="poolb")
        nc.scalar.copy(poolb[:], pooled[:])
        pp = psmisc.tile([1, B], FP32, tag="pm")
        for f in range(DF):
            nc.tensor.matmul(pp[:], fcw[:, f:f + 1], poolb[:, f],
                             start=(f == 0), stop=(f == DF - 1))
        outt = sb.tile([1, B], FP32, tag="outt")
        nc.scalar.copy(outt[:], pp[:])
        nc.scalar.dma_start(out_par[:], outt[:])
        es.close()
    nc.finalize()
    return nc


_CACHED = {}


def kernel(**inputs):
    inputs = {k: np.asarray(v) for k, v in inputs.items()}
    ids = np.asarray(inputs["input_ids"])
    rand_blocks = np.asarray(inputs["rand_blocks"])
    bf = lambda a: np.ascontiguousarray(a).astype(ml_dtypes.bfloat16)
    f32 = lambda a: np.ascontiguousarray(a, dtype=np.float32)

    n = NB
    rows = np.arange(n)
    allowed = np.zeros((H, n, n), bool)
    for h in range(H):
        allowed[h, rows, 0] = True
        allowed[h, rows, n - 1] = True
        allowed[h, rows, np.clip(rows - 1, 0, n - 1)] = True
        allowed[h, rows, rows] = True
        allowed[h, rows, np.clip(rows + 1, 0, n - 1)] = True
        for r in range(R):
            allowed[h, rows, rand_blocks[h, :, r]] = True
    allowed[:, 0, :] = True
    allowed[:, n - 1, :] = True

    emb = np.asarray(inputs["emb_tok"])[ids]      # [2, 4096, 768]
    pos = np.asarray(inputs["emb_pos"])
    kblk = np.arange(S) // BLK

    shared = {"fcw": bf(np.asarray(inputs["fc_w"])[:, 0].reshape(DF, P).T)}
    for nm in ("Wq", "Wk", "Wv", "Wo"):
        shared[nm] = bf(inputs[nm].reshape(L, DF, P, D))
    shared["W1"] = bf(inputs["W1"].reshape(L, DF, P, FF))
    shared["W2"] = bf(inputs["W2"].reshape(L, DFF, P, D))
    for nm in ("bq", "bk", "bv", "bo", "b2"):
        shared[nm] = f32(inputs[nm].reshape(L, DF, P).transpose(0, 2, 1))
    shared["b1"] = f32(inputs["b1"].reshape(L, DFF, P).transpose(0, 2, 1))
    lng = np.stack([inputs["ln_emb_g"], inputs["ln1_g"][0], inputs["ln2_g"][0],
                    inputs["ln1_g"][1], inputs["ln2_g"][1]])
    lnb = np.stack([inputs["ln_emb_b"], inputs["ln1_b"][0], inputs["ln2_b"][0],
                    inputs["ln1_b"][1], inputs["ln2_b"][1]])
    shared["lng"] = f32(lng.reshape(5, DF, P).transpose(0, 2, 1))
    shared["lnb"] = f32(lnb.reshape(5, DF, P).transpose(0, 2, 1))

    in_maps = []
    for c in range(NC):
        g = c % 4
        sl = slice(512 * c, 512 * (c + 1))
        embT = f32(np.concatenate([emb[0, sl], emb[1, sl]], 0).T
                   .reshape(DF, P, TPC))
        posT = f32(np.concatenate([pos[sl], pos[sl]], 0).T.reshape(DF, P, TPC))
        m = allowed[3 * g:3 * g + 3][:, :, kblk]          # [3, ib, k]
        m01 = bf(m.transpose(0, 2, 1).reshape(3, NK, P, NB)
                 .transpose(0, 2, 1, 3))
        in_maps.append({"embT": embT, "posT": posT, "m01": m01, **shared})

    if "nc" not in _CACHED:
        _CACHED["nc"] = build()
    res = run_bass_kernel_spmd(_CACHED["nc"], in_maps, list(range(NC))).results
    partial = np.stack([r["partial"][0] for r in res])
    out = partial.sum(0) / S + np.asarray(inputs["fc_b"])[0]
    return out.astype(np.float32)
